# revision 5
# baseline (speedup 1.0000x reference)
"""AttnDecoderRNN Trainium2 kernel: 8-core SPMD.

Sharding: data-parallel recurrence over batch (4/core), AllGather of hidden
states, vocab-sharded output projection (6400 cols/core, bf16 weights resident
in SBUF), on-device log_softmax via AllReduce of per-shard sum-exp stats.
"""

import numpy as np
import ml_dtypes

import concourse.bass as bass
import concourse.tile as tile
from concourse import bacc, mybir
from concourse.bass_utils import run_bass_kernel_spmd

F32 = mybir.dt.float32
BF16 = mybir.dt.bfloat16
AF = mybir.ActivationFunctionType
ALU = mybir.AluOpType
AX = mybir.AxisListType

NCORES = 8
B, S, H, V = 32, 50, 256, 50257
T = 25
BL = B // NCORES          # 4 batches per core
VS = 6400                 # padded vocab shard per core (8*6400 = 51200 >= V)
VPAD = VS * NCORES
ROWS = T * B              # 800 rows of (core, t, b_local)
RL = T * BL               # 100 rows produced per core
HT = H // 128             # 2 partition tiles for H
MT = (ROWS + 127) // 128  # 7 row tiles
NT = (VS + 511) // 512    # 13 vocab col tiles (12x512 + 256)


def _build(nc):
    d = {}

    def din(name, shape, dt=F32):
        d[name] = nc.dram_tensor(name, shape, dt, kind="ExternalInput").ap()
        return d[name]

    def dout(name, shape, dt=F32):
        d[name] = nc.dram_tensor(name, shape, dt, kind="ExternalOutput").ap()
        return d[name]

    keysT = din("keysT", [HT, 128, BL * S])
    embT = din("embT", [HT, 128, RL])
    h0T = din("h0T", [HT, 128, BL])
    h0B = din("h0B", [BL, H])
    ident = din("ident", [128, 128])
    WaT = din("WaT", [HT, 128, H])
    Wab = din("Wab", [HT, 128, 1])
    UaT = din("UaT", [HT, 128, H])
    Uab = din("Uab", [HT, 128, 1])
    Vac = din("Vac", [HT, 128, 1])
    WihT = din("WihT", [2 * HT, 128, 3 * H])
    WhhT = din("WhhT", [HT, 128, 3 * H])
    bih = din("bih", [1, 3 * H])
    bhh = din("bhh", [1, 3 * H])
    owt = din("owt", [HT, 128, VS], BF16)
    outb = din("outb", [1, VS], BF16)

    dec = dout("dec", [ROWS, VS])
    attn_d = dout("attn", [T, BL, S])
    hfin = dout("hfin", [BL, H])

    with tile.TileContext(nc) as tc:
        with (
            tc.tile_pool(name="const", bufs=1) as cp,
            tc.tile_pool(name="dram", bufs=1, space="DRAM") as dp,
            tc.tile_pool(name="step", bufs=1) as sp,
            tc.tile_pool(name="hb", bufs=3) as hbp,
        ):
            # ---- load constants ----
            keys_sb = cp.tile([128, HT, BL * S], F32)
            emb_sb = cp.tile([128, HT, RL], F32)
            h0T_sb = cp.tile([128, HT, BL], F32)
            h0B_sb = cp.tile([BL, H], F32)
            ident_sb = cp.tile([128, 128], F32)
            WaT_sb = cp.tile([128, HT, H], F32)
            Wab_sb = cp.tile([128, HT, 1], F32)
            UaT_sb = cp.tile([128, HT, H], F32)
            Uab_sb = cp.tile([128, HT, 1], F32)
            Va_sb = cp.tile([128, HT, 1], F32)
            Wih_sb = cp.tile([128, 2 * HT, 3 * H], F32)
            Whh_sb = cp.tile([128, HT, 3 * H], F32)
            bih_sb = cp.tile([1, 3 * H], F32)
            bhh_sb = cp.tile([1, 3 * H], F32)
            owt_sb = cp.tile([128, HT, VS], BF16)
            outb_sb = cp.tile([1, VS], BF16)
            ones_f = cp.tile([1, 128], F32)
            ones_b = cp.tile([1, 128], BF16)

            for i in range(HT):
                nc.sync.dma_start(keys_sb[:, i, :], keysT[i])
                nc.sync.dma_start(emb_sb[:, i, :], embT[i])
                nc.sync.dma_start(h0T_sb[:, i, :], h0T[i])
                nc.sync.dma_start(WaT_sb[:, i, :], WaT[i])
                nc.sync.dma_start(Wab_sb[:, i, :], Wab[i])
                nc.sync.dma_start(UaT_sb[:, i, :], UaT[i])
                nc.sync.dma_start(Uab_sb[:, i, :], Uab[i])
                nc.sync.dma_start(Va_sb[:, i, :], Vac[i])
                nc.sync.dma_start(Whh_sb[:, i, :], WhhT[i])
                nc.sync.dma_start(owt_sb[:, i, :], owt[i])
            for i in range(2 * HT):
                nc.sync.dma_start(Wih_sb[:, i, :], WihT[i])
            nc.sync.dma_start(h0B_sb[:], h0B[:])
            nc.sync.dma_start(ident_sb[:], ident[:])
            nc.sync.dma_start(bih_sb[:], bih[:])
            nc.sync.dma_start(bhh_sb[:], bhh[:])
            nc.sync.dma_start(outb_sb[:], outb[:])
            nc.gpsimd.memset(ones_f[:], 1.0)
            nc.gpsimd.memset(ones_b[:], 1.0)

            # ---- ua_keys = Ua @ keys + Ua_b   [g, (bl,s)] ----
            ua_sb = cp.tile([128, HT, BL * S], F32)
            rec_pools = (
                tc.tile_pool(name="pwa", bufs=1, space="PSUM"),
                tc.tile_pool(name="psc", bufs=1, space="PSUM"),
                tc.tile_pool(name="pwb", bufs=1, space="PSUM"),
                tc.tile_pool(name="pgi", bufs=1, space="PSUM"),
                tc.tile_pool(name="pgh", bufs=1, space="PSUM"),
                tc.tile_pool(name="ptr", bufs=1, space="PSUM"),
            )
            pwa_p, psc_p, pwb_p, pgi_p, pgh_p, ptr_p = (
                p.__enter__() for p in rec_pools)
            if True:
                for gt in range(HT):
                    p_ua = pwb_p.tile([128, BL * S], F32, tag="p_wb")
                    for kt in range(HT):
                        nc.tensor.matmul(
                            p_ua[:],
                            UaT_sb[:, kt, bass.ts(gt, 128)],
                            keys_sb[:, kt, :],
                            start=(kt == 0), stop=(kt == HT - 1),
                        )
                    nc.scalar.activation(ua_sb[:, gt, :], p_ua[:], AF.Identity,
                                         bias=Uab_sb[:, gt, :])

            # ---- recurrence ----
            h2all = cp.tile([128, HT, RL], F32)
            attn_sb = cp.tile([1, T, BL, S], F32)
            hB_prev = h0B_sb
            for t in range(T):
                def hT(i, _t=t):
                    if _t == 0:
                        return h0T_sb[:, i, :]
                    return h2all[:, i, bass.ts(_t - 1, BL)]

                # wa_q = Wa @ h + Wa_b  -> [g, bl]
                p_wa = pwa_p.tile([128, HT * BL], F32)
                wa_sb = sp.tile([128, HT, BL], F32, tag="wa")
                for gt in range(HT):
                    for kt in range(HT):
                        nc.tensor.matmul(
                            p_wa[:, bass.ts(gt, BL)],
                            WaT_sb[:, kt, bass.ts(gt, 128)],
                            hT(kt),
                            start=(kt == 0), stop=(kt == HT - 1),
                        )
                    nc.scalar.activation(wa_sb[:, gt, :], p_wa[:, bass.ts(gt, BL)],
                                         AF.Identity, bias=Wab_sb[:, gt, :])

                # e = tanh(wa_q + ua_keys) -> [g, (bl,s)]
                e_sb = sp.tile([128, HT, BL * S], F32, tag="e")
                for gt in range(HT):
                    for bl in range(BL):
                        nc.scalar.activation(
                            e_sb[:, gt, bass.ts(bl, S)],
                            ua_sb[:, gt, bass.ts(bl, S)],
                            AF.Tanh, bias=wa_sb[:, gt, bl:bl + 1],
                        )

                # scores = Va . e -> [1, (bl,s)]
                p_sc = psc_p.tile([1, BL * S], F32)
                for gt in range(HT):
                    nc.tensor.matmul(p_sc[:], Va_sb[:, gt, :], e_sb[:, gt, :],
                                     start=(gt == 0), stop=(gt == HT - 1))

                # softmax over s (no max-sub; scores are bounded)
                aslc = attn_sb[0:1, t, :, :]
                nc.scalar.activation(
                    aslc, p_sc[:].rearrange("p (b s) -> p b s", s=S), AF.Exp)
                ssum = sp.tile([1, BL], F32, tag="ssum")
                nc.vector.tensor_reduce(ssum[:], aslc, AX.X, ALU.add)
                srec = sp.tile([1, BL], F32, tag="srec")
                nc.vector.reciprocal(srec[:], ssum[:])
                for bl in range(BL):
                    nc.vector.tensor_scalar_mul(
                        attn_sb[0:1, t, bl, :], attn_sb[0:1, t, bl, :],
                        srec[0:1, bl:bl + 1])

                # broadcast w to 128 partitions via PE, then ctx
                p_wb = pwb_p.tile([128, BL * S], F32, tag="p_wb")
                nc.tensor.matmul(
                    p_wb[:], ones_f[:, :],
                    attn_sb[0:1, t, :, :].rearrange("p b s -> p (b s)"),
                    start=True, stop=True)
                wb_sb = sp.tile([128, BL * S], F32, tag="wb")
                nc.vector.tensor_copy(wb_sb[:], p_wb[:])
                ctx_sb = sp.tile([128, HT, BL], F32, tag="ctx")
                for gt in range(HT):
                    prod = sp.tile([128, BL, S], F32, tag="prod")
                    nc.vector.tensor_mul(
                        prod[:],
                        keys_sb[:, gt, :].rearrange("p (b s) -> p b s", s=S),
                        wb_sb[:].rearrange("p (b s) -> p b s", s=S))
                    nc.vector.tensor_reduce(ctx_sb[:, gt, :], prod[:], AX.X, ALU.add)

                # GRU gates: gi = W_ih @ [emb; ctx] + b_ih ; gh = W_hh @ h + b_hh
                p_gi = pgi_p.tile([BL, 3 * H], F32)
                p_gh = pgh_p.tile([BL, 3 * H], F32)
                nsl = [(0, 512), (512, 256)]
                for n0, nw in nsl:
                    for kt in range(2 * HT):
                        x_kt = (emb_sb[:, kt, bass.ts(t, BL)] if kt < HT
                                else ctx_sb[:, kt - HT, :])
                        nc.tensor.matmul(p_gi[:, n0:n0 + nw], x_kt,
                                         Wih_sb[:, kt, n0:n0 + nw],
                                         start=(kt == 0), stop=False)
                    nc.tensor.matmul(p_gi[:, n0:n0 + nw], ones_f[:, :BL],
                                     bih_sb[:, n0:n0 + nw], start=False, stop=True)
                    for kt in range(HT):
                        nc.tensor.matmul(p_gh[:, n0:n0 + nw], hT(kt),
                                         Whh_sb[:, kt, n0:n0 + nw],
                                         start=(kt == 0), stop=False)
                    nc.tensor.matmul(p_gh[:, n0:n0 + nw], ones_f[:, :BL],
                                     bhh_sb[:, n0:n0 + nw], start=False, stop=True)

                ghs = sp.tile([BL, 3 * H], F32, tag="ghs")
                nc.scalar.activation(ghs[:], p_gh[:], AF.Copy)
                rg = sp.tile([BL, H], F32, tag="rg")
                zg = sp.tile([BL, H], F32, tag="zg")
                ng = sp.tile([BL, H], F32, tag="ng")
                tmp = sp.tile([BL, H], F32, tag="tmp")
                nc.vector.tensor_add(tmp[:], p_gi[:, 0:H], ghs[:, 0:H])
                nc.scalar.activation(rg[:], tmp[:], AF.Sigmoid)
                tmp2 = sp.tile([BL, H], F32, tag="tmp2")
                nc.vector.tensor_add(tmp2[:], p_gi[:, H:2 * H], ghs[:, H:2 * H])
                nc.scalar.activation(zg[:], tmp2[:], AF.Sigmoid)
                tmp3 = sp.tile([BL, H], F32, tag="tmp3")
                nc.vector.tensor_mul(tmp3[:], rg[:], ghs[:, 2 * H:3 * H])
                nc.vector.tensor_add(tmp3[:], p_gi[:, 2 * H:3 * H], tmp3[:])
                nc.scalar.activation(ng[:], tmp3[:], AF.Tanh)
                # h2 = n + z * (h - n)
                h2B = hbp.tile([BL, H], F32, tag="h2b")
                nc.vector.tensor_sub(tmp[:], hB_prev[:], ng[:])
                nc.vector.tensor_mul(tmp[:], zg[:], tmp[:])
                nc.vector.tensor_add(h2B[:], ng[:], tmp[:])
                hB_prev = h2B

                # transpose h2 [BL,H] -> [H,BL] into h2all
                p_tr = ptr_p.tile([128, 2 * BL], F32)
                for i in range(HT):
                    nc.tensor.transpose(p_tr[:, bass.ts(i, BL)],
                                        h2B[:, bass.ts(i, 128)],
                                        ident_sb[:BL, :BL])
                    nc.scalar.activation(h2all[:, i, bass.ts(t, BL)],
                                         p_tr[:, bass.ts(i, BL)], AF.Copy)

            for p in reversed(rec_pools):
                p.__exit__(None, None, None)
            nc.sync.dma_start(hfin[:], h2B[:])
            nc.sync.dma_start(attn_d[:], attn_sb[0:1, :, :, :])

            # ---- AllGather h2 across cores ----
            ccin = dp.tile([HT, 128, RL], F32)
            ccout = dp.tile([NCORES * HT, 128, RL], F32)
            for i in range(HT):
                nc.sync.dma_start(ccin[i], h2all[:, i, :])
            nc.gpsimd.collective_compute(
                "AllGather", ALU.bypass,
                replica_groups=[list(range(NCORES))],
                ins=[ccin.opt()], outs=[ccout.opt()])

            # ---- vocab-sharded projection + log_softmax stats ----
            sums_sb = cp.tile([128, MT, NT], F32)
            logits_sb = cp.tile([128, MT, NT * 512], BF16)
            nc.gpsimd.memset(sums_sb[:], 0.0)
            with (
                tc.tile_pool(name="lt", bufs=2) as ltp,
                tc.tile_pool(name="plog", bufs=4, space="PSUM") as plp,
                tc.tile_pool(name="ex", bufs=2) as exp_p,
            ):
                for mt in range(MT):
                    r0 = mt * 128
                    mw = min(128, ROWS - r0)
                    lf = ltp.tile([128, HT, 128], F32, tag="lf")
                    for c in range(r0 // RL, (r0 + mw - 1) // RL + 1):
                        a, b = max(r0, c * RL), min(r0 + mw, (c + 1) * RL)
                        for i in range(HT):
                            nc.sync.dma_start(
                                lf[:, i, a - r0:b - r0],
                                ccout[c * HT + i][:, a - c * RL:b - c * RL])
                    lb = ltp.tile([128, HT, 128], BF16, tag="lb")
                    nc.vector.tensor_copy(lb[:, :, :mw], lf[:, :, :mw])
                    for nt in range(NT):
                        n0 = nt * 512
                        nw = min(512, VS - n0)
                        p_log = plp.tile([128, 512], F32)
                        for kt in range(HT):
                            nc.tensor.matmul(p_log[:mw, :nw], lb[:, kt, :mw],
                                             owt_sb[:, kt, n0:n0 + nw],
                                             start=(kt == 0), stop=False)
                        nc.tensor.matmul(p_log[:mw, :nw], ones_b[:, :mw],
                                         outb_sb[:, n0:n0 + nw],
                                         start=False, stop=True)
                        nc.vector.tensor_copy(
                            logits_sb[:mw, mt, n0:n0 + nw], p_log[:mw, :nw])
                        ext = exp_p.tile([128, 512], BF16, tag="ext")
                        nc.scalar.activation(
                            ext[:mw, :nw], p_log[:mw, :nw], AF.Exp,
                            accum_out=sums_sb[:mw, mt, nt:nt + 1])

            # ---- AllReduce sum-exp, lse = ln(sum), subtract, write out ----
            sloc = cp.tile([128, MT], F32)
            nc.vector.tensor_reduce(sloc[:], sums_sb[:], AX.X, ALU.add)
            cc2in = dp.tile([128, MT], F32)
            cc2out = dp.tile([128, MT], F32)
            nc.sync.dma_start(cc2in[:], sloc[:])
            nc.gpsimd.collective_compute(
                "AllReduce", ALU.add,
                replica_groups=[list(range(NCORES))],
                ins=[cc2in.opt()], outs=[cc2out.opt()])
            sg = cp.tile([128, MT], F32)
            nc.sync.dma_start(sg[:], cc2out[:])
            lse = cp.tile([128, MT], F32)
            nc.scalar.activation(lse[:], sg[:], AF.Ln)

            with tc.tile_pool(name="of", bufs=4) as ofp:
                for mt in range(MT):
                    r0 = mt * 128
                    mw = min(128, ROWS - r0)
                    for nt in range(NT):
                        n0 = nt * 512
                        nw = min(512, VS - n0)
                        of = ofp.tile([128, 512], F32, tag="of")
                        nc.vector.tensor_scalar(
                            of[:mw, :nw], logits_sb[:mw, mt, n0:n0 + nw],
                            lse[:mw, mt:mt + 1], None, ALU.subtract)
                        nc.sync.dma_start(dec[r0:r0 + mw, n0:n0 + nw],
                                          of[:mw, :nw])
    return d


_CACHE = {}


def _get_compiled():
    if "nc" not in _CACHE:
        nc = bacc.Bacc("TRN2", target_bir_lowering=False, debug=False,
                       enable_asserts=False, num_devices=NCORES)
        _build(nc)
        nc.compile()
        _CACHE["nc"] = nc
    return _CACHE["nc"]


def make_in_maps(encoder_hidden, encoder_outputs, target_tensor, embedding,
                 Wa_w, Wa_b, Ua_w, Ua_b, Va_w, Va_b,
                 gru_w_ih, gru_b_ih, gru_w_hh, gru_b_hh, out_w, out_b):
    f = np.float32
    toks = np.concatenate(
        [np.zeros((B, 1), dtype=np.int64),
         np.asarray(target_tensor)[:, :T - 1].astype(np.int64)], axis=1).T
    emb_seq = np.asarray(embedding, f)[toks]           # [T, B, H]
    owtT = np.zeros((H, VPAD), f)
    owtT[:, :V] = np.asarray(out_w, f).T
    outb_p = np.full((VPAD,), -1e30, f)
    outb_p[:V] = np.asarray(out_b, f)
    owtT_b = owtT.astype(ml_dtypes.bfloat16)
    outb_b = outb_p.astype(ml_dtypes.bfloat16)
    h0 = np.asarray(encoder_hidden, f)[0]              # [B, H]
    keys = np.asarray(encoder_outputs, f)              # [B, S, H]

    common = {
        "ident": np.eye(128, dtype=f),
        "WaT": np.asarray(Wa_w, f).T.reshape(HT, 128, H),
        "Wab": np.asarray(Wa_b, f).reshape(HT, 128, 1),
        "UaT": np.asarray(Ua_w, f).T.reshape(HT, 128, H),
        "Uab": np.asarray(Ua_b, f).reshape(HT, 128, 1),
        "Vac": np.asarray(Va_w, f)[0].reshape(HT, 128, 1),
        "WihT": np.asarray(gru_w_ih, f).T.reshape(2 * HT, 128, 3 * H),
        "WhhT": np.asarray(gru_w_hh, f).T.reshape(HT, 128, 3 * H),
        "bih": np.asarray(gru_b_ih, f).reshape(1, 3 * H),
        "bhh": np.asarray(gru_b_hh, f).reshape(1, 3 * H),
    }
    in_maps = []
    for c in range(NCORES):
        bs = slice(c * BL, (c + 1) * BL)
        m = dict(common)
        m["keysT"] = np.ascontiguousarray(
            keys[bs].transpose(2, 0, 1).reshape(HT, 128, BL * S))
        m["embT"] = np.ascontiguousarray(
            emb_seq[:, bs].transpose(2, 0, 1).reshape(HT, 128, RL))
        m["h0T"] = np.ascontiguousarray(h0[bs].T.reshape(HT, 128, BL))
        m["h0B"] = np.ascontiguousarray(h0[bs])
        m["owt"] = np.ascontiguousarray(
            owtT_b[:, c * VS:(c + 1) * VS].reshape(HT, 128, VS))
        m["outb"] = outb_b[c * VS:(c + 1) * VS].reshape(1, VS)
        in_maps.append(m)
    return in_maps


def kernel(**inputs):
    nc = _get_compiled()
    in_maps = make_in_maps(**inputs)
    res = run_bass_kernel_spmd(nc, in_maps, core_ids=list(range(NCORES)))
    outs = res.results
    # dec: rows r = c_src*RL + t*BL + bl ; cols = vocab shard per core
    dec_full = np.concatenate([outs[c]["dec"] for c in range(NCORES)], axis=1)
    dec_full = dec_full[:, :V].reshape(NCORES, T, BL, V)
    dec_out = dec_full.transpose(0, 2, 1, 3).reshape(B, T, V)
    attn = np.concatenate(
        [outs[c]["attn"].reshape(1, T, BL, S) for c in range(NCORES)], axis=0)
    attn = attn.transpose(0, 2, 1, 3).reshape(B, T, S)
    hT_out = np.concatenate(
        [outs[c]["hfin"] for c in range(NCORES)], axis=0)[None]
    return (np.asarray(dec_out, np.float32), np.asarray(hT_out, np.float32),
            np.asarray(attn, np.float32))
